# revision 38
# baseline (speedup 1.0000x reference)
"""PlanStack kernel for Trainium2 (8 NeuronCores, batch-sharded SPMD).

Math (B=16384, D=8, H=1024, per-row):
  push   = LayerNorm(hidden @ W_push.T + b_push) * ln_g + ln_b
  is_pop = (hidden @ w_pop.T + b_pop) > 0
  ptr    = int(pointer);  do_pop = is_pop & ptr>0;  do_push = ~is_pop & ptr<D
  prev   = stack[ptr-1] (0 if ptr==0)
  top    = do_push ? push : prev          (prev already 0 when ptr==0)
  popped = do_pop ? prev : 0
  new_ptr = ptr + do_push - do_pop
  new_stack = stack with push written at slot ptr where do_push
"""

import os
import time
import numpy as np
from contextlib import ExitStack

B, D, H = 16384, 8, 1024
NCORES = 8
RB = B // NCORES          # 2048 rows per core
P = 128                   # partitions
NT = RB // P              # 16 tiles per core
EPS = 1e-5

_last_exec_ns = None


def _build(neg_bpop: float, trivial_ln: bool):
    from concourse import bass, tile
    from concourse import mybir
    from concourse.bacc import Bacc
    from concourse.masks import make_identity

    f32 = mybir.dt.float32
    Alu = mybir.AluOpType
    Act = mybir.ActivationFunctionType

    # Bacc (not plain Bass): its finalize() runs the legalization pipeline
    # (move_matmul_waits_to_ldweights, generate_event_semaphores) that splits
    # multi-sem waits down to the 1-wait-per-instruction TRN2 ISA limit.
    # Without it walrus codegen fails with "Too many sync wait commands".
    nc = Bacc()

    hs_d = nc.declare_dram_parameter("hidden_state", [RB, H], f32, isOutput=False)
    st_d = nc.declare_dram_parameter("stack", [RB, D, H], f32, isOutput=False)
    pt_d = nc.declare_dram_parameter("pointer", [RB, 1], f32, isOutput=False)
    wt_d = nc.declare_dram_parameter("W_push_t", [H, H], f32, isOutput=False)
    bp_d = nc.declare_dram_parameter("b_push2", [1, H], f32, isOutput=False)
    wp_d = nc.declare_dram_parameter("w_pop_t", [H, 1], f32, isOutput=False)
    if not trivial_ln:
        lng_d = nc.declare_dram_parameter("ln_g2", [1, H], f32, isOutput=False)
        lnb_d = nc.declare_dram_parameter("ln_b2", [1, H], f32, isOutput=False)

    ns_d = nc.declare_dram_parameter("new_stack", [RB, D, H], f32, isOutput=True)
    np_d = nc.declare_dram_parameter("new_pointer", [RB, 1], f32, isOutput=True)
    top_d = nc.declare_dram_parameter("top", [RB, H], f32, isOutput=True)
    pop_d = nc.declare_dram_parameter("popped", [RB, H], f32, isOutput=True)

    with tile.TileContext(nc) as tc, ExitStack() as ctx:
        consts = ctx.enter_context(tc.tile_pool(name="consts", bufs=1))
        trp = ctx.enter_context(tc.tile_pool(name="trp", bufs=2, space="PSUM"))
        pp_pool = ctx.enter_context(tc.tile_pool(name="pp", bufs=2, space="PSUM"))
        lg_pool = ctx.enter_context(tc.tile_pool(name="lg", bufs=2, space="PSUM"))

        stk_pool = ctx.enter_context(tc.tile_pool(name="stk", bufs=3))
        hid_pool = ctx.enter_context(tc.tile_pool(name="hid", bufs=2))
        ht_pool = ctx.enter_context(tc.tile_pool(name="ht", bufs=2))
        push_pool = ctx.enter_context(tc.tile_pool(name="push", bufs=2))
        prev_pool = ctx.enter_context(tc.tile_pool(name="prev", bufs=2))
        top_pool = ctx.enter_context(tc.tile_pool(name="topp", bufs=2))
        popd_pool = ctx.enter_context(tc.tile_pool(name="popd", bufs=2))
        small = ctx.enter_context(tc.tile_pool(name="small", bufs=2))

        # ---- constants ----
        ident = consts.tile([P, P], f32, name="ident", tag="ident")
        make_identity(nc, ident)
        ones = consts.tile([1, P], f32, name="ones", tag="ones")
        nc.gpsimd.memset(ones[:], 1.0)
        dvals_i = consts.tile([P, D], mybir.dt.int32, name="dvals_i", tag="dvals_i")
        nc.gpsimd.iota(dvals_i[:], pattern=[[1, D]], base=0, channel_multiplier=0)
        dvals = consts.tile([P, D], f32, name="dvals", tag="dvals")
        nc.vector.tensor_copy(out=dvals[:], in_=dvals_i[:])
        dvals1 = consts.tile([P, D], f32, name="dvals1", tag="dvals1")
        nc.vector.tensor_scalar_add(out=dvals1[:], in0=dvals[:], scalar1=1.0)
        epst = consts.tile([P, 1], f32, name="epst", tag="epst")
        nc.gpsimd.memset(epst[:], EPS)

        wt = consts.tile([P, D, H], f32, name="wt", tag="wt")  # wt[:,c,:] = W^T chunk c
        for c in range(8):
            nc.sync.dma_start(out=wt[:, c, :], in_=wt_d[c * P:(c + 1) * P, :])
        wpT = consts.tile([P, D, 1], f32, name="wpT", tag="wpT")
        for c in range(8):
            nc.scalar.dma_start(out=wpT[:, c, :], in_=wp_d[c * P:(c + 1) * P, :])
        bpush = consts.tile([1, H], f32, name="bpush", tag="bpush")
        nc.scalar.dma_start(out=bpush[:], in_=bp_d[:])
        if not trivial_ln:
            lng = consts.tile([P, H], f32, name="lng", tag="lng")
            nc.sync.dma_start(out=lng[:], in_=lng_d[:].to_broadcast((P, H)))
            lnb = consts.tile([P, H], f32, name="lnb", tag="lnb")
            nc.sync.dma_start(out=lnb[:], in_=lnb_d[:].to_broadcast((P, H)))

        for t in range(NT):
            r0, r1 = t * P, (t + 1) * P

            # Only SP (sync), ACT (scalar) and Pool (gpsimd SWDGE) can issue
            # DMAs; each engine owns one dynamic queue (~42GB/s each measured).
            # Slot-splits keep a per-row DRAM discontinuity so each DMA breaks
            # into 128 descriptors that parallelize across the queue's engines;
            # fully-contiguous DRAM transfers serialize to ~12GB/s (measured).
            stk = stk_pool.tile([P, D, H], f32, name="stk", tag="stk")
            nc.sync.dma_start(out=stk[:, 0:5, :], in_=st_d[r0:r1, 0:5, :])
            nc.scalar.dma_start(out=stk[:, 5:8, :], in_=st_d[r0:r1, 5:8, :])
            hid = hid_pool.tile([P, H], f32, name="hid", tag="hid")
            nc.gpsimd.dma_start(out=hid[:], in_=hs_d[r0:r1])
            ptrt = small.tile([P, 1], f32, name="ptrt", tag="ptrt")
            nc.scalar.dma_start(out=ptrt[:], in_=pt_d[r0:r1])

            # hidden^T via PE transpose, 8 chunks of [128,128]
            ht = ht_pool.tile([P, H], f32, name="ht", tag="ht")
            for c in range(8):
                trt = trp.tile([P, P], f32, name="trt", tag="trt")
                nc.tensor.transpose(out=trt[:], in_=hid[:, c * P:(c + 1) * P],
                                    identity=ident[:])
                nc.scalar.activation(out=ht[:, c * P:(c + 1) * P], in_=trt[:],
                                     func=Act.Copy, bias=0.0, scale=1.0)

            # matmuls: push = hid @ W^T + b_push ; logit = hid @ w_pop^T
            pp = pp_pool.tile([P, H], f32, name="pp", tag="pp")
            lg = lg_pool.tile([P, 1], f32, name="lg", tag="lg")
            for c in range(8):
                lhs = ht[:, c * P:(c + 1) * P]
                nc.tensor.matmul(out=pp[:, 0:512], lhsT=lhs, rhs=wt[:, c, 0:512],
                                 start=(c == 0), stop=False)
                nc.tensor.matmul(out=pp[:, 512:1024], lhsT=lhs, rhs=wt[:, c, 512:1024],
                                 start=(c == 0), stop=False)
                nc.tensor.matmul(out=lg[:], lhsT=lhs, rhs=wpT[:, c, :],
                                 start=(c == 0), stop=(c == 7))
            nc.tensor.matmul(out=pp[:, 0:512], lhsT=ones[:], rhs=bpush[:, 0:512],
                             start=False, stop=True)
            nc.tensor.matmul(out=pp[:, 512:1024], lhsT=ones[:], rhs=bpush[:, 512:1024],
                             start=False, stop=True)

            # LayerNorm over H
            stats = small.tile([P, 2, 6], f32, name="stats", tag="stats")
            nc.vector.bn_stats(out=stats[:, 0, :], in_=pp[:, 0:512])
            nc.vector.bn_stats(out=stats[:, 1, :], in_=pp[:, 512:1024])
            mv = small.tile([P, 2], f32, name="mv", tag="mv")
            nc.vector.bn_aggr(out=mv[:], in_=stats[:])
            inv = small.tile([P, 1], f32, name="inv", tag="inv")
            nc.scalar.activation(out=inv[:], in_=mv[:, 1:2], func=Act.Sqrt,
                                 bias=epst[:], scale=1.0)
            nc.vector.reciprocal(out=inv[:], in_=inv[:])
            push = push_pool.tile([P, H], f32, name="push", tag="push")
            nc.vector.tensor_scalar(out=push[:], in0=pp[:], scalar1=mv[:, 0:1],
                                    scalar2=inv[:], op0=Alu.subtract, op1=Alu.mult)
            if not trivial_ln:
                nc.vector.tensor_tensor(out=push[:], in0=push[:], in1=lng[:],
                                        op=Alu.mult)
                nc.vector.tensor_tensor(out=push[:], in0=push[:], in1=lnb[:],
                                        op=Alu.add)

            # masks
            ptrlt = small.tile([P, 1], f32, name="ptrlt", tag="ptrlt")
            nc.vector.tensor_scalar(out=ptrlt[:], in0=ptrt[:], scalar1=float(D),
                                    scalar2=None, op0=Alu.is_lt)
            ptrgt = small.tile([P, 1], f32, name="ptrgt", tag="ptrgt")
            nc.vector.tensor_scalar(out=ptrgt[:], in0=ptrt[:], scalar1=0.0,
                                    scalar2=None, op0=Alu.is_gt)
            mpush = small.tile([P, 1], f32, name="mpush", tag="mpush")
            nc.vector.scalar_tensor_tensor(out=mpush[:], in0=lg[:], scalar=neg_bpop,
                                           in1=ptrlt[:], op0=Alu.is_le, op1=Alu.mult)
            mpop = small.tile([P, 1], f32, name="mpop", tag="mpop")
            nc.vector.scalar_tensor_tensor(out=mpop[:], in0=lg[:], scalar=neg_bpop,
                                           in1=ptrgt[:], op0=Alu.is_gt, op1=Alu.mult)
            selpush = small.tile([P, D], f32, name="selpush", tag="selpush")
            nc.vector.tensor_tensor(out=selpush[:], in0=ptrt[:].to_broadcast((P, D)),
                                    in1=dvals[:], op=Alu.is_equal)
            nc.vector.tensor_scalar_mul(out=selpush[:], in0=selpush[:],
                                        scalar1=mpush[:])
            i8 = mybir.dt.int8
            selpush8 = small.tile([P, D], i8, name="selpush8", tag="selpush8")
            nc.vector.tensor_copy(out=selpush8[:], in_=selpush[:])
            mpush8 = small.tile([P, 1], i8, name="mpush8", tag="mpush8")
            nc.vector.tensor_copy(out=mpush8[:], in_=mpush[:])

            # new pointer
            nptr = small.tile([P, 1], f32, name="nptr", tag="nptr")
            nc.vector.tensor_scalar(out=nptr[:], in0=ptrt[:], scalar1=mpush[:],
                                    scalar2=mpop[:], op0=Alu.add, op1=Alu.subtract)
            nc.scalar.dma_start(out=np_d[r0:r1], in_=nptr[:])

            # prev = stack[ptr-1] (0 when ptr==0): selprev[:,d] = (d+1 == ptr).
            # Slot 0 seeds prev via a mul (zeroes non-matching rows), slots
            # 1..7 overlay with copy_predicated.
            selprev = small.tile([P, D], f32, name="selprev", tag="selprev")
            nc.vector.tensor_tensor(out=selprev[:], in0=ptrt[:].to_broadcast((P, D)),
                                    in1=dvals1[:], op=Alu.is_equal)
            selprev8 = small.tile([P, D], i8, name="selprev8", tag="selprev8")
            nc.vector.tensor_copy(out=selprev8[:], in_=selprev[:])
            prev = prev_pool.tile([P, H], f32, name="prev", tag="prev")
            nc.vector.tensor_scalar_mul(out=prev[:], in0=stk[:, 0, :],
                                        scalar1=selprev[:, 0:1])
            for d in range(1, D):
                nc.vector.copy_predicated(
                    out=prev[:], mask=selprev8[:, d:d + 1].to_broadcast((P, H)),
                    data=stk[:, d, :])

            # top = mpush ? push : prev   (prev already 0 when ptr==0)
            topt = top_pool.tile([P, H], f32, name="topt", tag="topt")
            nc.scalar.activation(out=topt[:], in_=prev[:], func=Act.Copy,
                                 bias=0.0, scale=1.0)
            nc.vector.copy_predicated(out=topt[:], mask=mpush8[:].to_broadcast((P, H)),
                                      data=push[:])
            nc.sync.dma_start(out=top_d[r0:r1], in_=topt[:])

            # popped = mpop * prev  (mpop is 0 when ptr==0)
            popt = popd_pool.tile([P, H], f32, name="popt", tag="popt")
            nc.scalar.activation(out=popt[:], in_=prev[:], func=Act.Copy,
                                 bias=0.0, scale=mpop[:])
            nc.gpsimd.dma_start(out=pop_d[r0:r1], in_=popt[:])

            # scatter push into stack slots, then write back
            for d in range(D):
                nc.vector.copy_predicated(
                    out=stk[:, d, :], mask=selpush8[:, d:d + 1].to_broadcast((P, H)),
                    data=push[:])
            nc.scalar.dma_start(out=ns_d[r0:r1, 0:4, :], in_=stk[:, 0:4, :])
            nc.gpsimd.dma_start(out=ns_d[r0:r1, 4:8, :], in_=stk[:, 4:8, :])

    nc.finalize()
    return nc


def _run_timed(nc, in_maps, n_cores, iters=30):
    """Like bass2jax.run_bass_via_pjrt, but without output donation so the
    jitted executable can be re-run on device-resident inputs to measure
    per-iteration execution time (NTFF profiling is unavailable here)."""
    import jax
    from jax.sharding import Mesh, PartitionSpec, NamedSharding
    from jax.experimental.shard_map import shard_map
    from concourse import bass2jax, mybir

    bass2jax.install_neuronx_cc_hook()

    partition_name = nc.partition_id_tensor.name if nc.partition_id_tensor else None
    in_names, out_names, out_avals, zero_outs = [], [], [], []
    for alloc in nc.m.functions[0].allocations:
        if not isinstance(alloc, mybir.MemoryLocationSet):
            continue
        name = alloc.memorylocations[0].name
        if alloc.kind == "ExternalInput":
            if name != partition_name:
                in_names.append(name)
        elif alloc.kind == "ExternalOutput":
            out_names.append(name)
            shape = tuple(alloc.tensor_shape)
            dtype = mybir.dt.np(alloc.dtype)
            out_avals.append(jax.core.ShapedArray(shape, dtype))
            zero_outs.append(np.zeros(shape, dtype))
    n_params = len(in_names)
    n_outs = len(out_names)
    in_names.extend(out_names)
    if partition_name is not None:
        in_names.append(partition_name)

    def _body(*args):
        operands = list(args)
        if partition_name is not None:
            operands.append(bass2jax.partition_id_tensor())
        outs = bass2jax._bass_exec_p.bind(
            *operands,
            out_avals=tuple(out_avals),
            in_names=tuple(in_names),
            out_names=tuple(out_names),
            lowering_input_output_aliases=(),
            sim_require_finite=True,
            sim_require_nnan=True,
            nc=nc,
        )
        return tuple(outs)

    devices = jax.devices()[:n_cores]
    mesh = Mesh(np.asarray(devices), ("core",))
    in_specs = (PartitionSpec("core"),) * (n_params + n_outs)
    out_specs = (PartitionSpec("core"),) * n_outs
    fn = jax.jit(
        shard_map(_body, mesh=mesh, in_specs=in_specs, out_specs=out_specs,
                  check_rep=False),
        keep_unused=True,
    )

    per_core = [[np.asarray(m[name]) for name in in_names[:n_params]]
                for m in in_maps]
    concat_in = [
        np.concatenate([per_core[c][i] for c in range(n_cores)], axis=0)
        for i in range(n_params)
    ]
    concat_zeros = [
        np.zeros((n_cores * z.shape[0], *z.shape[1:]), z.dtype) for z in zero_outs
    ]
    sh = NamedSharding(mesh, PartitionSpec("core"))
    dev_in = [jax.device_put(a, sh) for a in concat_in + concat_zeros]

    iters = int(os.environ.get("KERNEL_ITERS", iters))
    out_arrs = fn(*dev_in)
    jax.block_until_ready(out_arrs)
    for _ in range(3):
        jax.block_until_ready(fn(*dev_in))
    # single-call latency (block after each)
    t0 = time.perf_counter_ns()
    for _ in range(5):
        jax.block_until_ready(fn(*dev_in))
    t1 = time.perf_counter_ns()
    single_ns = (t1 - t0) // 5
    # pipelined: enqueue all, block once
    t0 = time.perf_counter_ns()
    for _ in range(iters):
        r = fn(*dev_in)
    t_enq = time.perf_counter_ns()
    jax.block_until_ready(r)
    t1 = time.perf_counter_ns()
    per_iter_ns = (t1 - t0) // iters
    print(f"[timing] single-call {single_ns} ns, pipelined/iter {per_iter_ns} ns "
          f"(enqueue {(t_enq - t0) // iters} ns/iter, iters={iters})")

    results = [
        {
            name: np.asarray(out_arrs[i]).reshape(n_cores, *out_avals[i].shape)[c]
            for i, name in enumerate(out_names)
        }
        for c in range(n_cores)
    ]
    return results, per_iter_ns


def kernel(hidden_state, stack, pointer, W_push, b_push, ln_g, ln_b, w_pop, b_pop):
    global _last_exec_ns
    from concourse.bass_utils import run_bass_kernel_spmd

    f = np.float32
    hidden_state = np.ascontiguousarray(hidden_state, dtype=f)
    stack = np.ascontiguousarray(stack, dtype=f)
    pointer = np.ascontiguousarray(pointer, dtype=f)
    W_push_t = np.ascontiguousarray(np.asarray(W_push, dtype=f).T)
    b_push2 = np.ascontiguousarray(np.asarray(b_push, dtype=f).reshape(1, H))
    w_pop_t = np.ascontiguousarray(np.asarray(w_pop, dtype=f).reshape(H, 1))
    neg_bpop = -float(np.asarray(b_pop, dtype=f).reshape(-1)[0])
    ln_g = np.asarray(ln_g, dtype=f)
    ln_b = np.asarray(ln_b, dtype=f)
    trivial_ln = bool(np.all(ln_g == 1.0) and np.all(ln_b == 0.0))

    nc = _build(neg_bpop, trivial_ln)

    in_maps = []
    for i in range(NCORES):
        s = slice(i * RB, (i + 1) * RB)
        m = {
            "hidden_state": hidden_state[s],
            "stack": stack[s],
            "pointer": pointer[s],
            "W_push_t": W_push_t,
            "b_push2": b_push2,
            "w_pop_t": w_pop_t,
        }
        if not trivial_ln:
            m["ln_g2"] = ln_g.reshape(1, H)
            m["ln_b2"] = ln_b.reshape(1, H)
        in_maps.append(m)

    if os.environ.get("KERNEL_TIME"):
        results, _last_exec_ns = _run_timed(nc, in_maps, NCORES)
    else:
        res = run_bass_kernel_spmd(nc, in_maps, list(range(NCORES)))
        _last_exec_ns = res.exec_time_ns
        results = res.results

    new_stack = np.concatenate([results[i]["new_stack"] for i in range(NCORES)], axis=0)
    new_pointer = np.concatenate([results[i]["new_pointer"] for i in range(NCORES)], axis=0)
    top = np.concatenate([results[i]["top"] for i in range(NCORES)], axis=0)
    popped = np.concatenate([results[i]["popped"] for i in range(NCORES)], axis=0)
    return new_stack, new_pointer, top, popped


# revision 54
# speedup vs baseline: 1.0496x; 1.0496x over previous
"""PlanStack kernel for Trainium2 (8 NeuronCores, batch-sharded SPMD).

Math (B=16384, D=8, H=1024, per-row):
  push   = LayerNorm(hidden @ W_push.T + b_push) * ln_g + ln_b
  is_pop = (hidden @ w_pop.T + b_pop) > 0
  ptr    = int(pointer);  do_pop = is_pop & ptr>0;  do_push = ~is_pop & ptr<D
  prev   = stack[ptr-1] (0 if ptr==0)
  top    = do_push ? push : prev          (prev already 0 when ptr==0)
  popped = do_pop ? prev : 0
  new_ptr = ptr + do_push - do_pop
  new_stack = stack with push written at slot ptr where do_push
"""

import os
import time
import numpy as np
from contextlib import ExitStack

B, D, H = 16384, 8, 1024
NCORES = 8
RB = B // NCORES          # 2048 rows per core
P = 128                   # partitions
NT = RB // P              # 16 tiles per core
EPS = 1e-5

_last_exec_ns = None


def _build(neg_bpop: float, trivial_ln: bool):
    from concourse import bass, tile
    from concourse import mybir
    from concourse.bacc import Bacc
    from concourse.masks import make_identity

    f32 = mybir.dt.float32
    Alu = mybir.AluOpType
    Act = mybir.ActivationFunctionType

    # Bacc (not plain Bass): its finalize() runs the legalization pipeline
    # (move_matmul_waits_to_ldweights, generate_event_semaphores) that splits
    # multi-sem waits down to the 1-wait-per-instruction TRN2 ISA limit.
    # Without it walrus codegen fails with "Too many sync wait commands".
    nc = Bacc()

    hs_d = nc.declare_dram_parameter("hidden_state", [RB, H], f32, isOutput=False)
    st_d = nc.declare_dram_parameter("stack", [RB, D, H], f32, isOutput=False)
    pt_d = nc.declare_dram_parameter("pointer", [RB, 1], f32, isOutput=False)
    wt_d = nc.declare_dram_parameter("W_push_t", [H, H], f32, isOutput=False)
    bp_d = nc.declare_dram_parameter("b_push2", [1, H], f32, isOutput=False)
    wp_d = nc.declare_dram_parameter("w_pop_t", [H, 1], f32, isOutput=False)
    if not trivial_ln:
        lng_d = nc.declare_dram_parameter("ln_g2", [1, H], f32, isOutput=False)
        lnb_d = nc.declare_dram_parameter("ln_b2", [1, H], f32, isOutput=False)

    ns_d = nc.declare_dram_parameter("new_stack", [RB, D, H], f32, isOutput=True)
    np_d = nc.declare_dram_parameter("new_pointer", [RB, 1], f32, isOutput=True)
    top_d = nc.declare_dram_parameter("top", [RB, H], f32, isOutput=True)
    pop_d = nc.declare_dram_parameter("popped", [RB, H], f32, isOutput=True)

    with tile.TileContext(nc) as tc, ExitStack() as ctx:
        consts = ctx.enter_context(tc.tile_pool(name="consts", bufs=1))
        trp = ctx.enter_context(tc.tile_pool(name="trp", bufs=2, space="PSUM"))
        pp_pool = ctx.enter_context(tc.tile_pool(name="pp", bufs=2, space="PSUM"))
        lg_pool = ctx.enter_context(tc.tile_pool(name="lg", bufs=2, space="PSUM"))

        stk_pool = ctx.enter_context(tc.tile_pool(name="stk", bufs=3))
        hid_pool = ctx.enter_context(tc.tile_pool(name="hid", bufs=2))
        ht_pool = ctx.enter_context(tc.tile_pool(name="ht", bufs=2))
        push_pool = ctx.enter_context(tc.tile_pool(name="push", bufs=2))
        prev_pool = ctx.enter_context(tc.tile_pool(name="prev", bufs=2))
        top_pool = ctx.enter_context(tc.tile_pool(name="topp", bufs=2))
        popd_pool = ctx.enter_context(tc.tile_pool(name="popd", bufs=2))
        small = ctx.enter_context(tc.tile_pool(name="small", bufs=2))

        # ---- constants ----
        ident = consts.tile([P, P], f32, name="ident", tag="ident")
        make_identity(nc, ident)
        ones = consts.tile([1, P], f32, name="ones", tag="ones")
        nc.gpsimd.memset(ones[:], 1.0)
        dvals_i = consts.tile([P, D], mybir.dt.int32, name="dvals_i", tag="dvals_i")
        nc.gpsimd.iota(dvals_i[:], pattern=[[1, D]], base=0, channel_multiplier=0)
        dvals = consts.tile([P, D], f32, name="dvals", tag="dvals")
        nc.vector.tensor_copy(out=dvals[:], in_=dvals_i[:])
        dvals1 = consts.tile([P, D], f32, name="dvals1", tag="dvals1")
        nc.vector.tensor_scalar_add(out=dvals1[:], in0=dvals[:], scalar1=1.0)
        epst = consts.tile([P, 1], f32, name="epst", tag="epst")
        nc.gpsimd.memset(epst[:], EPS)

        wt = consts.tile([P, D, H], f32, name="wt", tag="wt")  # wt[:,c,:] = W^T chunk c
        for c in range(8):
            nc.sync.dma_start(out=wt[:, c, :], in_=wt_d[c * P:(c + 1) * P, :])
        wpT = consts.tile([P, D, 1], f32, name="wpT", tag="wpT")
        for c in range(8):
            nc.scalar.dma_start(out=wpT[:, c, :], in_=wp_d[c * P:(c + 1) * P, :])
        bpush = consts.tile([1, H], f32, name="bpush", tag="bpush")
        nc.scalar.dma_start(out=bpush[:], in_=bp_d[:])
        if not trivial_ln:
            lng = consts.tile([P, H], f32, name="lng", tag="lng")
            nc.sync.dma_start(out=lng[:], in_=lng_d[:].to_broadcast((P, H)))
            lnb = consts.tile([P, H], f32, name="lnb", tag="lnb")
            nc.sync.dma_start(out=lnb[:], in_=lnb_d[:].to_broadcast((P, H)))

        for t in range(NT):
            r0, r1 = t * P, (t + 1) * P

            # Only SP (sync), ACT (scalar) and Pool (gpsimd SWDGE) can issue
            # DMAs; each engine owns one dynamic queue (~42GB/s each measured).
            # Slot-splits keep a per-row DRAM discontinuity so each DMA breaks
            # into 128 descriptors that parallelize across the queue's engines;
            # fully-contiguous DRAM transfers serialize to ~12GB/s (measured).
            stk = stk_pool.tile([P, D, H], f32, name="stk", tag="stk")
            nc.sync.dma_start(out=stk[:, 0:5, :], in_=st_d[r0:r1, 0:5, :])
            nc.scalar.dma_start(out=stk[:, 5:8, :], in_=st_d[r0:r1, 5:8, :])
            hid = hid_pool.tile([P, H], f32, name="hid", tag="hid")
            nc.gpsimd.dma_start(out=hid[:], in_=hs_d[r0:r1])
            ptrt = small.tile([P, 1], f32, name="ptrt", tag="ptrt")
            nc.scalar.dma_start(out=ptrt[:], in_=pt_d[r0:r1])

            # hidden^T via PE transpose, 8 chunks of [128,128]
            ht = ht_pool.tile([P, H], f32, name="ht", tag="ht")
            for c in range(8):
                trt = trp.tile([P, P], f32, name="trt", tag="trt")
                nc.tensor.transpose(out=trt[:], in_=hid[:, c * P:(c + 1) * P],
                                    identity=ident[:])
                nc.scalar.activation(out=ht[:, c * P:(c + 1) * P], in_=trt[:],
                                     func=Act.Copy, bias=0.0, scale=1.0)

            # matmuls: push = hid @ W^T + b_push ; logit = hid @ w_pop^T
            pp = pp_pool.tile([P, H], f32, name="pp", tag="pp")
            lg = lg_pool.tile([P, 1], f32, name="lg", tag="lg")
            for c in range(8):
                lhs = ht[:, c * P:(c + 1) * P]
                nc.tensor.matmul(out=pp[:, 0:512], lhsT=lhs, rhs=wt[:, c, 0:512],
                                 start=(c == 0), stop=False)
                nc.tensor.matmul(out=pp[:, 512:1024], lhsT=lhs, rhs=wt[:, c, 512:1024],
                                 start=(c == 0), stop=False)
                nc.tensor.matmul(out=lg[:], lhsT=lhs, rhs=wpT[:, c, :],
                                 start=(c == 0), stop=(c == 7))
            nc.tensor.matmul(out=pp[:, 0:512], lhsT=ones[:], rhs=bpush[:, 0:512],
                             start=False, stop=True)
            nc.tensor.matmul(out=pp[:, 512:1024], lhsT=ones[:], rhs=bpush[:, 512:1024],
                             start=False, stop=True)

            # LayerNorm over H
            stats = small.tile([P, 2, 6], f32, name="stats", tag="stats")
            nc.vector.bn_stats(out=stats[:, 0, :], in_=pp[:, 0:512])
            nc.vector.bn_stats(out=stats[:, 1, :], in_=pp[:, 512:1024])
            mv = small.tile([P, 2], f32, name="mv", tag="mv")
            nc.vector.bn_aggr(out=mv[:], in_=stats[:])
            inv = small.tile([P, 1], f32, name="inv", tag="inv")
            nc.scalar.activation(out=inv[:], in_=mv[:, 1:2], func=Act.Sqrt,
                                 bias=epst[:], scale=1.0)
            nc.vector.reciprocal(out=inv[:], in_=inv[:])
            push = push_pool.tile([P, H], f32, name="push", tag="push")
            nc.vector.tensor_scalar(out=push[:], in0=pp[:], scalar1=mv[:, 0:1],
                                    scalar2=inv[:], op0=Alu.subtract, op1=Alu.mult)
            if not trivial_ln:
                nc.vector.tensor_tensor(out=push[:], in0=push[:], in1=lng[:],
                                        op=Alu.mult)
                nc.vector.tensor_tensor(out=push[:], in0=push[:], in1=lnb[:],
                                        op=Alu.add)

            # masks
            ptrlt = small.tile([P, 1], f32, name="ptrlt", tag="ptrlt")
            nc.vector.tensor_scalar(out=ptrlt[:], in0=ptrt[:], scalar1=float(D),
                                    scalar2=None, op0=Alu.is_lt)
            ptrgt = small.tile([P, 1], f32, name="ptrgt", tag="ptrgt")
            nc.vector.tensor_scalar(out=ptrgt[:], in0=ptrt[:], scalar1=0.0,
                                    scalar2=None, op0=Alu.is_gt)
            mpush = small.tile([P, 1], f32, name="mpush", tag="mpush")
            nc.vector.scalar_tensor_tensor(out=mpush[:], in0=lg[:], scalar=neg_bpop,
                                           in1=ptrlt[:], op0=Alu.is_le, op1=Alu.mult)
            mpop = small.tile([P, 1], f32, name="mpop", tag="mpop")
            nc.vector.scalar_tensor_tensor(out=mpop[:], in0=lg[:], scalar=neg_bpop,
                                           in1=ptrgt[:], op0=Alu.is_gt, op1=Alu.mult)
            selpush = small.tile([P, D], f32, name="selpush", tag="selpush")
            nc.vector.tensor_tensor(out=selpush[:], in0=ptrt[:].to_broadcast((P, D)),
                                    in1=dvals[:], op=Alu.is_equal)
            nc.vector.tensor_scalar_mul(out=selpush[:], in0=selpush[:],
                                        scalar1=mpush[:])
            i8 = mybir.dt.int8
            selpush8 = small.tile([P, D], i8, name="selpush8", tag="selpush8")
            nc.vector.tensor_copy(out=selpush8[:], in_=selpush[:])
            mpush8 = small.tile([P, 1], i8, name="mpush8", tag="mpush8")
            nc.vector.tensor_copy(out=mpush8[:], in_=mpush[:])

            # new pointer
            nptr = small.tile([P, 1], f32, name="nptr", tag="nptr")
            nc.vector.tensor_scalar(out=nptr[:], in0=ptrt[:], scalar1=mpush[:],
                                    scalar2=mpop[:], op0=Alu.add, op1=Alu.subtract)
            nc.scalar.dma_start(out=np_d[r0:r1], in_=nptr[:])

            # prev = stack[ptr-1] (0 when ptr==0): selprev[:,d] = (d+1 == ptr).
            # Slot 0 seeds prev via a mul (zeroes non-matching rows), slots
            # 1..7 overlay with copy_predicated.
            selprev = small.tile([P, D], f32, name="selprev", tag="selprev")
            nc.vector.tensor_tensor(out=selprev[:], in0=ptrt[:].to_broadcast((P, D)),
                                    in1=dvals1[:], op=Alu.is_equal)
            selprev8 = small.tile([P, D], i8, name="selprev8", tag="selprev8")
            nc.vector.tensor_copy(out=selprev8[:], in_=selprev[:])
            prev = prev_pool.tile([P, H], f32, name="prev", tag="prev")
            nc.vector.tensor_scalar_mul(out=prev[:], in0=stk[:, 0, :],
                                        scalar1=selprev[:, 0:1])
            for d in range(1, D):
                nc.vector.copy_predicated(
                    out=prev[:], mask=selprev8[:, d:d + 1].to_broadcast((P, H)),
                    data=stk[:, d, :])

            # top = mpush ? push : prev   (prev already 0 when ptr==0)
            topt = top_pool.tile([P, H], f32, name="topt", tag="topt")
            nc.scalar.activation(out=topt[:], in_=prev[:], func=Act.Copy,
                                 bias=0.0, scale=1.0)
            nc.vector.copy_predicated(out=topt[:], mask=mpush8[:].to_broadcast((P, H)),
                                      data=push[:])
            nc.sync.dma_start(out=top_d[r0:r1], in_=topt[:])

            # popped = mpop * prev  (mpop is 0 when ptr==0)
            popt = popd_pool.tile([P, H], f32, name="popt", tag="popt")
            nc.scalar.activation(out=popt[:], in_=prev[:], func=Act.Copy,
                                 bias=0.0, scale=mpop[:])
            nc.gpsimd.dma_start(out=pop_d[r0:r1], in_=popt[:])

            # scatter push into stack slots, then write back
            for d in range(D):
                nc.vector.copy_predicated(
                    out=stk[:, d, :], mask=selpush8[:, d:d + 1].to_broadcast((P, H)),
                    data=push[:])
            nc.scalar.dma_start(out=ns_d[r0:r1, 0:4, :], in_=stk[:, 0:4, :])
            nc.gpsimd.dma_start(out=ns_d[r0:r1, 4:8, :], in_=stk[:, 4:8, :])

    nc.finalize()
    return nc


def _run_timed(nc, in_maps, n_cores, iters=30):
    """Like bass2jax.run_bass_via_pjrt, but without output donation so the
    jitted executable can be re-run on device-resident inputs to measure
    per-iteration execution time (NTFF profiling is unavailable here)."""
    import jax
    from jax.sharding import Mesh, PartitionSpec, NamedSharding
    from jax.experimental.shard_map import shard_map
    from concourse import bass2jax, mybir

    bass2jax.install_neuronx_cc_hook()

    partition_name = nc.partition_id_tensor.name if nc.partition_id_tensor else None
    in_names, out_names, out_avals, zero_outs = [], [], [], []
    for alloc in nc.m.functions[0].allocations:
        if not isinstance(alloc, mybir.MemoryLocationSet):
            continue
        name = alloc.memorylocations[0].name
        if alloc.kind == "ExternalInput":
            if name != partition_name:
                in_names.append(name)
        elif alloc.kind == "ExternalOutput":
            out_names.append(name)
            shape = tuple(alloc.tensor_shape)
            dtype = mybir.dt.np(alloc.dtype)
            out_avals.append(jax.core.ShapedArray(shape, dtype))
            zero_outs.append(np.zeros(shape, dtype))
    n_params = len(in_names)
    n_outs = len(out_names)
    in_names.extend(out_names)
    if partition_name is not None:
        in_names.append(partition_name)

    def _body(*args):
        operands = list(args)
        if partition_name is not None:
            operands.append(bass2jax.partition_id_tensor())
        outs = bass2jax._bass_exec_p.bind(
            *operands,
            out_avals=tuple(out_avals),
            in_names=tuple(in_names),
            out_names=tuple(out_names),
            lowering_input_output_aliases=(),
            sim_require_finite=True,
            sim_require_nnan=True,
            nc=nc,
        )
        return tuple(outs)

    devices = jax.devices()[:n_cores]
    mesh = Mesh(np.asarray(devices), ("core",))
    in_specs = (PartitionSpec("core"),) * (n_params + n_outs)
    out_specs = (PartitionSpec("core"),) * n_outs
    fn = jax.jit(
        shard_map(_body, mesh=mesh, in_specs=in_specs, out_specs=out_specs,
                  check_rep=False),
        keep_unused=True,
    )

    per_core = [[np.asarray(m[name]) for name in in_names[:n_params]]
                for m in in_maps]
    concat_in = [
        np.concatenate([per_core[c][i] for c in range(n_cores)], axis=0)
        for i in range(n_params)
    ]
    concat_zeros = [
        np.zeros((n_cores * z.shape[0], *z.shape[1:]), z.dtype) for z in zero_outs
    ]
    sh = NamedSharding(mesh, PartitionSpec("core"))
    dev_in = [jax.device_put(a, sh) for a in concat_in + concat_zeros]

    iters = int(os.environ.get("KERNEL_ITERS", iters))
    out_arrs = fn(*dev_in)
    jax.block_until_ready(out_arrs)
    for _ in range(3):
        jax.block_until_ready(fn(*dev_in))
    # single-call latency (block after each)
    t0 = time.perf_counter_ns()
    for _ in range(5):
        jax.block_until_ready(fn(*dev_in))
    t1 = time.perf_counter_ns()
    single_ns = (t1 - t0) // 5
    # pipelined: enqueue all, block once
    t0 = time.perf_counter_ns()
    for _ in range(iters):
        r = fn(*dev_in)
    t_enq = time.perf_counter_ns()
    jax.block_until_ready(r)
    t1 = time.perf_counter_ns()
    per_iter_ns = (t1 - t0) // iters
    print(f"[timing] single-call {single_ns} ns, pipelined/iter {per_iter_ns} ns "
          f"(enqueue {(t_enq - t0) // iters} ns/iter, iters={iters})")

    results = [
        {
            name: np.asarray(out_arrs[i]).reshape(n_cores, *out_avals[i].shape)[c]
            for i, name in enumerate(out_names)
        }
        for c in range(n_cores)
    ]
    return results, per_iter_ns


def kernel(hidden_state, stack, pointer, W_push, b_push, ln_g, ln_b, w_pop, b_pop):
    global _last_exec_ns
    from concourse.bass_utils import run_bass_kernel_spmd

    f = np.float32
    hidden_state = np.ascontiguousarray(hidden_state, dtype=f)
    stack = np.ascontiguousarray(stack, dtype=f)
    pointer = np.ascontiguousarray(pointer, dtype=f)
    W_push_t = np.ascontiguousarray(np.asarray(W_push, dtype=f).T)
    b_push2 = np.ascontiguousarray(np.asarray(b_push, dtype=f).reshape(1, H))
    w_pop_t = np.ascontiguousarray(np.asarray(w_pop, dtype=f).reshape(H, 1))
    neg_bpop = -float(np.asarray(b_pop, dtype=f).reshape(-1)[0])
    ln_g = np.asarray(ln_g, dtype=f)
    ln_b = np.asarray(ln_b, dtype=f)
    trivial_ln = bool(np.all(ln_g == 1.0) and np.all(ln_b == 0.0))

    nc = _build(neg_bpop, trivial_ln)

    in_maps = []
    for i in range(NCORES):
        s = slice(i * RB, (i + 1) * RB)
        m = {
            "hidden_state": hidden_state[s],
            "stack": stack[s],
            "pointer": pointer[s],
            "W_push_t": W_push_t,
            "b_push2": b_push2,
            "w_pop_t": w_pop_t,
        }
        if not trivial_ln:
            m["ln_g2"] = ln_g.reshape(1, H)
            m["ln_b2"] = ln_b.reshape(1, H)
        in_maps.append(m)

    if os.environ.get("KERNEL_TIME"):
        results, _last_exec_ns = _run_timed(nc, in_maps, NCORES)
    else:
        res = run_bass_kernel_spmd(nc, in_maps, list(range(NCORES)))
        _last_exec_ns = res.exec_time_ns
        results = res.results

    new_stack = np.concatenate([results[i]["new_stack"] for i in range(NCORES)], axis=0)
    new_pointer = np.concatenate([results[i]["new_pointer"] for i in range(NCORES)], axis=0)
    top = np.concatenate([results[i]["top"] for i in range(NCORES)], axis=0)
    popped = np.concatenate([results[i]["popped"] for i in range(NCORES)], axis=0)
    return new_stack, new_pointer, top, popped
